# revision 17
# baseline (speedup 1.0000x reference)
"""TRN2 Bass kernel for nn_LinearBinary: out = (A @ W + b) +/- 1 per-row.

    A: [8192, 2048] f32, W: [2048, 2048] f32, b: [2048] f32
    C = A @ W + b;  cond = C[:, :1] > 0.5;  out = where(cond, C+1, C-1)

Sharding: data-parallel over the 8192-row batch across 8 NeuronCores
(1024 rows/core); W and b replicated. SPMD - one program, per-core shards
via in_maps.

Per-core kernel (v3 — PE-minimal, DMA-tuned):
  - W shipped as bf16 (8MB) and kept fully resident in SBUF as 4
    ko-groups, streamed on the sync (SP) HWDGE queue with 4KB packets.
  - A^T shipped pre-transposed bf16 from the host, blocked per m-pair as
    [pair, kp, ko, m] so each partition line is one contiguous 8KB run:
    DMA arbitration is packet-round-robin, so packet size IS bandwidth
    share — 512B A^T packets get starved 8:1 by 4KB W packets.
  - PE does ZERO transposes — only the 512 bf16 matmuls (1 cyc/row,
    262144 cycles ~= 109us at 2.4GHz).
  - m-tiles in PAIRS across all 8 PSUM banks. Pair 0 runs ko-group-outer
    so its matmuls stream right behind the W group DMAs; pairs 1-3 run
    ko-inner per psum so each [128,512] tile closes early and its
    epilogue+store overlaps the remaining matmuls (kills the end-of-pair
    store burst and the final tail).
  - The row condition needs exact fp32 C[:, 0] (min |C0-0.5| margin on
    this data is ~4.4e-4; bf16 A would flip rows): computed on
    gpsimd (mult) + DVE (reduce) from the natural-layout fp32 A.
  - Epilogue fuses (psum + (-+1)) + b in one scalar_tensor_tensor per
    [128, 512] tile; stores go on the sync queue (idle once W landed).
"""

import sys

for _p in ("/opt/trn_rl_repo", "/root/.axon_site/_ro/trn_rl_repo"):
    if _p not in sys.path:
        sys.path.append(_p)

import ml_dtypes
import numpy as np

import concourse.bacc as bacc
import concourse.mybir as mybir
import concourse.tile as tile
from concourse.bass_utils import run_bass_kernel_spmd

dt = mybir.dt
Alu = mybir.AluOpType
BF16 = np.dtype(ml_dtypes.bfloat16)

P = 128
K = 2048
N = 2048
B_FULL = 8192
N_CORES = 8
M_SHARD = B_FULL // N_CORES  # 1024 rows per core
M_TILES = M_SHARD // P  # 8
KO = K // P  # 16
KG = 8  # W ko-groups (DMA granularity)
KGS = KO // KG  # 2 ko per group
NQ = 4  # PSUM n-chunks
N_SUB = N // NQ  # 512
PAIRS = M_TILES // 2  # m-tiles processed 2 at a time (8 PSUM banks)
MP = 2 * P  # rows per pair


def _build():
    nc = bacc.Bacc("TRN2", target_bir_lowering=False, debug=False, num_devices=N_CORES)

    a = nc.dram_tensor("inputs", [M_SHARD, K], dt.float32, kind="ExternalInput")
    # A^T, host-blocked per m-pair: at[pr, kp, ko, mm] = A[pr*256+mm, ko*128+kp]
    at = nc.dram_tensor("at", [PAIRS, P, KO, MP], dt.bfloat16, kind="ExternalInput")
    # W host-blocked per ko-group: w[g, kp, kk, n] = W[(g*KGS+kk)*128+kp, n]
    # so each partition line is one contiguous 8KB run (big DMA packets win
    # the packet-round-robin arbitration).
    w = nc.dram_tensor("w", [KG, P, KGS, N], dt.bfloat16, kind="ExternalInput")
    b = nc.dram_tensor("b", [N], dt.float32, kind="ExternalInput")
    # W[:, 0] pre-sliced on host: a strided 4-byte column-gather DMA is fatal
    # on HW (NRT_EXEC_UNIT_UNRECOVERABLE), so ship the 8KB row directly.
    w0 = nc.dram_tensor("w0", [1, K], dt.float32, kind="ExternalInput")
    out = nc.dram_tensor("out", [M_SHARD, N], dt.float32, kind="ExternalOutput")

    with tile.TileContext(nc) as tc:
        with (
            tc.tile_pool(name="consts", bufs=1) as consts,
            tc.tile_pool(name="wg", bufs=1) as wg_pool,
            tc.tile_pool(name="atp", bufs=1) as at_pool,
            tc.tile_pool(name="anat", bufs=2) as anat_pool,
            tc.tile_pool(name="scr", bufs=2) as scr_pool,
            tc.tile_pool(name="dsm", bufs=1) as d_pool,
            tc.tile_pool(name="outs", bufs=4) as out_pool,
            tc.tile_pool(name="psc", bufs=8, space="PSUM") as psum_pool,
        ):
            # b and W[:, 0] broadcast to all partitions (scalar queue: tiny)
            b_row = consts.tile([1, N], dt.float32, tag="b_row")
            nc.scalar.dma_start(b_row[:], b.ap().unsqueeze(0))
            b128 = consts.tile([P, N], dt.float32, tag="b128")
            nc.gpsimd.partition_broadcast(b128[:], b_row[:])
            w0_row = consts.tile([1, K], dt.float32, tag="w0_row")
            nc.sync.dma_start(w0_row[:], w0.ap())
            w0b = consts.tile([P, K], dt.float32, tag="w0b")
            nc.gpsimd.partition_broadcast(w0b[:], w0_row[:])

            # warm-up: dummy matmuls on a zeroed bf16 tile while the first
            # DMAs land, so the PE p-state ramp happens off the critical path
            warm = consts.tile([P, P], dt.bfloat16, tag="warm")
            nc.vector.memset(warm[:], 0.0)
            ps_w = psum_pool.tile([P, N_SUB], dt.float32, tag="psum", name="ps_w")
            for _ in range(8):
                nc.tensor.matmul(
                    ps_w[:, :P], warm[:], warm[:], start=True, stop=True
                )

            # W resident, bf16, 8 ko-groups of 1MB. The bandwidth cap is
            # per-HWDGE-queue (~180GB/s each), so split W across BOTH
            # HWDGE queues; atp0 and g0 (which together gate the first
            # matmul) go FIRST on opposite queues. Pair 0 consumes groups
            # in merged arrival order CONSUME0 below.
            atps = []
            for pr in range(PAIRS):
                atp = at_pool.tile([P, KO, MP], dt.bfloat16, tag=f"atp{pr}", name=f"atp{pr}")
                atps.append(atp)

            def load_atp(pr, eng):
                eng.dma_start(atps[pr][:], at.ap()[pr])

            load_atp(0, nc.sync)

            wgs = [None] * KG
            issue = [(0, nc.scalar), (2, nc.sync), (1, nc.scalar), (3, nc.sync),
                     (4, nc.scalar), (6, nc.sync), (5, nc.scalar), (7, nc.scalar)]
            for g, eng in issue:
                wg = wg_pool.tile([P, KGS, N], dt.bfloat16, tag=f"wg{g}", name=f"wg{g}")
                eng.dma_start(wg[:], w.ap()[g])
                wgs[g] = wg
            # arrival: scalar g0@~14,g1@~19,g4@~25,g5@~30,g7@~36;
            # sync atp0@~14,g2@~19,g3@~25,g6@~30
            CONSUME0 = [0, 2, 1, 3, 4, 6, 5, 7]

            load_atp(1, nc.scalar)
            load_atp(2, nc.sync)
            load_atp(3, nc.sync)

            def cond_m(m, eng):
                # exact fp32 condition: d = (A[m-tile] @ w0 + b0 > 0.5) ? +1 : -1
                a_nat = anat_pool.tile([P, K], dt.float32, tag="a_nat", name="a_nat")
                eng.dma_start(a_nat[:], a.ap()[m * P : (m + 1) * P, :])
                scratch = scr_pool.tile([P, K], dt.float32, tag="scratch", name="scratch")
                c0 = d_pool.tile([P, 1], dt.float32, tag=f"c0_{m}", name=f"c0_{m}")
                nc.vector.tensor_tensor(scratch[:], a_nat[:], w0b[:], Alu.mult)
                nc.vector.tensor_reduce(c0[:], scratch[:], mybir.AxisListType.X, Alu.add)
                g = d_pool.tile([P, 1], dt.float32, tag=f"g_{m}", name=f"g_{m}")
                nc.vector.tensor_scalar(
                    g[:], c0[:], b128[:, 0:1], 0.5, Alu.add, Alu.is_gt
                )
                d = d_pool.tile([P, 1], dt.float32, tag=f"d_{m}", name=f"d_{m}")
                nc.vector.tensor_scalar(d[:], g[:], 2.0, -1.0, Alu.mult, Alu.add)
                return d

            def epilogue(psum, d, m, nq):
                out_sb = out_pool.tile([P, N_SUB], dt.float32, tag="out_sb", name="out_sb")
                nc.vector.scalar_tensor_tensor(
                    out_sb[:],
                    psum[:],
                    d[:],
                    b128[:, nq * N_SUB : (nq + 1) * N_SUB],
                    Alu.add,
                    Alu.add,
                )
                nc.sync.dma_start(
                    out.ap()[m * P : (m + 1) * P, nq * N_SUB : (nq + 1) * N_SUB],
                    out_sb[:],
                )

            # a_nat loads ride the gpsimd SWDGE queue (third DMA stream);
            # conditions are issued one pair AHEAD so the in-order DVE queue
            # never has a data-starved cond blocking the previous pair's
            # epilogues (PSUM WAR head-of-line).
            conds = {}
            conds[0] = cond_m(0, nc.gpsimd)
            conds[1] = cond_m(1, nc.gpsimd)

            for pr in range(PAIRS):
                m0 = 2 * pr
                if pr + 1 < PAIRS:
                    conds[m0 + 2] = cond_m(m0 + 2, nc.gpsimd)
                    conds[m0 + 3] = cond_m(m0 + 3, nc.gpsimd)
                ds = [conds[m0], conds[m0 + 1]]
                atp = atps[pr]

                if pr == 0:
                    # ko-group outer in DMA arrival order, streaming right
                    # behind the W group DMAs
                    psums = [
                        [
                            psum_pool.tile(
                                [P, N_SUB], dt.float32, name=f"ps_{mi}_{nq}", tag="psum"
                            )
                            for nq in range(NQ)
                        ]
                        for mi in range(2)
                    ]
                    for gi, g4 in enumerate(CONSUME0):
                        for kk in range(KGS):
                            ko = g4 * KGS + kk
                            for mi in range(2):
                                for nq in range(NQ):
                                    nc.tensor.matmul(
                                        psums[mi][nq][:],
                                        atp[:, ko, mi * P : (mi + 1) * P],
                                        wgs[g4][:, kk, nq * N_SUB : (nq + 1) * N_SUB],
                                        start=(gi == 0 and kk == 0),
                                        stop=(gi == KG - 1 and kk == KGS - 1),
                                    )
                    for mi in range(2):
                        for nq in range(NQ):
                            epilogue(psums[mi][nq], ds[mi], m0 + mi, nq)
                else:
                    # W fully resident: ko-inner per psum so each tile closes
                    # early and its epilogue+store overlaps remaining matmuls
                    for mi in range(2):
                        for nq in range(NQ):
                            psum = psum_pool.tile(
                                [P, N_SUB], dt.float32, name="ps", tag="psum"
                            )
                            for ko in range(KO):
                                nc.tensor.matmul(
                                    psum[:],
                                    atp[:, ko, mi * P : (mi + 1) * P],
                                    wgs[ko // KGS][:, ko % KGS, nq * N_SUB : (nq + 1) * N_SUB],
                                    start=(ko == 0),
                                    stop=(ko == KO - 1),
                                )
                            epilogue(psum, ds[mi], m0 + mi, nq)

    nc.compile()
    return nc


_NC = None


def _get_nc():
    global _NC
    if _NC is None:
        _NC = _build()
    return _NC


def build_in_maps(a, w, b):
    """Host-side prep: shard A, pre-transpose/block to bf16, cast W to bf16."""
    a = np.ascontiguousarray(a, dtype=np.float32)
    w = np.ascontiguousarray(w, dtype=np.float32)
    b = np.ascontiguousarray(b, dtype=np.float32)
    # w_blk[g, kp, kk, n] = W[(g*KGS+kk)*P + kp, n], bf16
    w_blk = np.ascontiguousarray(
        np.transpose(w.astype(BF16).reshape(KG, KGS, P, N), (0, 2, 1, 3))
    )
    w0 = np.ascontiguousarray(w[:, 0].reshape(1, K))
    in_maps = []
    for i in range(N_CORES):
        a_sh = a[i * M_SHARD : (i + 1) * M_SHARD]
        # at[pr, kp, ko, mm] = a_sh[pr*256+mm, ko*128+kp]
        at = np.transpose(
            a_sh.reshape(PAIRS, MP, KO, P), (0, 3, 2, 1)
        ).astype(BF16)
        in_maps.append(
            {"inputs": a_sh, "at": at, "w": w_blk, "b": b, "w0": w0}
        )
    return in_maps


def kernel(**inputs: np.ndarray) -> np.ndarray:
    a = inputs["inputs"]
    assert a.shape == (B_FULL, K), a.shape
    nc = _get_nc()
    in_maps = build_in_maps(a, inputs["w"], inputs["b"])
    res = run_bass_kernel_spmd(nc, in_maps, core_ids=list(range(N_CORES)))
    return np.concatenate([res.results[i]["out"] for i in range(N_CORES)], axis=0)


# revision 21
# speedup vs baseline: 1.0364x; 1.0364x over previous
"""TRN2 Bass kernel for nn_LinearBinary: out = (A @ W + b) +/- 1 per-row.

    A: [8192, 2048] f32, W: [2048, 2048] f32, b: [2048] f32
    C = A @ W + b;  cond = C[:, :1] > 0.5;  out = where(cond, C+1, C-1)

Sharding: data-parallel over the 8192-row batch across 8 NeuronCores
(1024 rows/core); W and b replicated. SPMD - one program, per-core shards
via in_maps.

Per-core kernel (v3 — PE-minimal, DMA-tuned):
  - W shipped as bf16 (8MB) and kept fully resident in SBUF as 4
    ko-groups, streamed on the sync (SP) HWDGE queue with 4KB packets.
  - A^T shipped pre-transposed bf16 from the host, blocked per m-pair as
    [pair, kp, ko, m] so each partition line is one contiguous 8KB run:
    DMA arbitration is packet-round-robin, so packet size IS bandwidth
    share — 512B A^T packets get starved 8:1 by 4KB W packets.
  - PE does ZERO transposes — only the 512 bf16 matmuls (1 cyc/row,
    262144 cycles ~= 109us at 2.4GHz).
  - m-tiles in PAIRS across all 8 PSUM banks. Pair 0 runs ko-group-outer
    so its matmuls stream right behind the W group DMAs; pairs 1-3 run
    ko-inner per psum so each [128,512] tile closes early and its
    epilogue+store overlaps the remaining matmuls (kills the end-of-pair
    store burst and the final tail).
  - The row condition needs exact fp32 C[:, 0] (min |C0-0.5| margin on
    this data is ~4.4e-4; bf16 A would flip rows): computed on
    gpsimd (mult) + DVE (reduce) from the natural-layout fp32 A.
  - Epilogue fuses (psum + (-+1)) + b in one scalar_tensor_tensor per
    [128, 512] tile; stores go on the sync queue (idle once W landed).
"""

import sys

for _p in ("/opt/trn_rl_repo", "/root/.axon_site/_ro/trn_rl_repo"):
    if _p not in sys.path:
        sys.path.append(_p)

import ml_dtypes
import numpy as np

import concourse.bacc as bacc
import concourse.mybir as mybir
import concourse.tile as tile
from concourse.bass_utils import run_bass_kernel_spmd

dt = mybir.dt
Alu = mybir.AluOpType
BF16 = np.dtype(ml_dtypes.bfloat16)

P = 128
K = 2048
N = 2048
B_FULL = 8192
N_CORES = 8
M_SHARD = B_FULL // N_CORES  # 1024 rows per core
M_TILES = M_SHARD // P  # 8
KO = K // P  # 16
KG = 8  # W ko-groups (DMA granularity)
KGS = KO // KG  # 2 ko per group
NQ = 4  # PSUM n-chunks
N_SUB = N // NQ  # 512
PAIRS = M_TILES // 2  # m-tiles processed 2 at a time (8 PSUM banks)
MP = 2 * P  # rows per pair


def _build():
    nc = bacc.Bacc("TRN2", target_bir_lowering=False, debug=False, num_devices=N_CORES)

    a = nc.dram_tensor("inputs", [M_SHARD, K], dt.float32, kind="ExternalInput")
    # A^T, host-blocked per m-pair: at[pr, kp, ko, mm] = A[pr*256+mm, ko*128+kp]
    at = nc.dram_tensor("at", [PAIRS, P, KO, MP], dt.bfloat16, kind="ExternalInput")
    # W host-blocked per ko-group: w[g, kp, kk, n] = W[(g*KGS+kk)*128+kp, n]
    # so each partition line is one contiguous 8KB run (big DMA packets win
    # the packet-round-robin arbitration).
    w = nc.dram_tensor("w", [KG, P, KGS, N], dt.bfloat16, kind="ExternalInput")
    b = nc.dram_tensor("b", [N], dt.float32, kind="ExternalInput")
    # W[:, 0] pre-sliced on host: a strided 4-byte column-gather DMA is fatal
    # on HW (NRT_EXEC_UNIT_UNRECOVERABLE), so ship the 8KB row directly.
    w0 = nc.dram_tensor("w0", [1, K], dt.float32, kind="ExternalInput")
    out = nc.dram_tensor("out", [M_SHARD, N], dt.float32, kind="ExternalOutput")

    with tile.TileContext(nc) as tc:
        with (
            tc.tile_pool(name="consts", bufs=1) as consts,
            tc.tile_pool(name="wg", bufs=1) as wg_pool,
            tc.tile_pool(name="atp", bufs=1) as at_pool,
            tc.tile_pool(name="anat", bufs=2) as anat_pool,
            tc.tile_pool(name="scr", bufs=2) as scr_pool,
            tc.tile_pool(name="dsm", bufs=1) as d_pool,
            tc.tile_pool(name="outs", bufs=4) as out_pool,
            tc.tile_pool(name="psc", bufs=8, space="PSUM") as psum_pool,
        ):
            # b and W[:, 0] broadcast to all partitions (scalar queue: tiny)
            b_row = consts.tile([1, N], dt.float32, tag="b_row")
            nc.scalar.dma_start(b_row[:], b.ap().unsqueeze(0))
            b128 = consts.tile([P, N], dt.float32, tag="b128")
            nc.gpsimd.partition_broadcast(b128[:], b_row[:])
            w0_row = consts.tile([1, K], dt.float32, tag="w0_row")
            nc.sync.dma_start(w0_row[:], w0.ap())
            w0b = consts.tile([P, K], dt.float32, tag="w0b")
            nc.gpsimd.partition_broadcast(w0b[:], w0_row[:])

            # warm-up: dummy matmuls on a zeroed bf16 tile while the first
            # DMAs land, so the PE p-state ramp happens off the critical path
            warm = consts.tile([P, P], dt.bfloat16, tag="warm")
            nc.vector.memset(warm[:], 0.0)
            ps_w = psum_pool.tile([P, N_SUB], dt.float32, tag="psum", name="ps_w")
            for _ in range(8):
                nc.tensor.matmul(
                    ps_w[:, :P], warm[:], warm[:], start=True, stop=True
                )

            # W resident, bf16, 8 ko-groups of 1MB. The bandwidth cap is
            # per-HWDGE-queue (~180GB/s each), so split W across BOTH
            # HWDGE queues; atp0 and g0 (which together gate the first
            # matmul) go FIRST on opposite queues. Pair 0 consumes groups
            # in merged arrival order CONSUME0 below.
            atps = []
            for pr in range(PAIRS):
                atp = at_pool.tile([P, KO, MP], dt.bfloat16, tag=f"atp{pr}", name=f"atp{pr}")
                atps.append(atp)

            def load_atp(pr, eng):
                eng.dma_start(atps[pr][:], at.ap()[pr])

            load_atp(0, nc.sync)

            wgs = [None] * KG
            issue = [(0, nc.scalar), (2, nc.sync), (1, nc.scalar), (3, nc.sync),
                     (4, nc.scalar), (6, nc.sync), (5, nc.scalar), (7, nc.scalar)]
            for g, eng in issue:
                wg = wg_pool.tile([P, KGS, N], dt.bfloat16, tag=f"wg{g}", name=f"wg{g}")
                eng.dma_start(wg[:], w.ap()[g])
                wgs[g] = wg
            # arrival: scalar g0@~14,g1@~19,g4@~25,g5@~30,g7@~36;
            # sync atp0@~14,g2@~19,g3@~25,g6@~30
            CONSUME0 = [0, 2, 1, 3, 4, 6, 5, 7]

            # exact fp32 condition: d = (A[m-tile] @ w0 + b0 > 0.5) ? +1 : -1
            # Split: DMA + multiply (gpsimd for m>=2, so a data-starved op
            # can never sit ahead of epilogue stts in the in-order DVE
            # queue), then reduce + compares on DVE issued pair-aligned.
            def cond_start(m, dma_eng, me):
                a_nat = anat_pool.tile([P, K], dt.float32, tag="a_nat", name="a_nat")
                dma_eng.dma_start(a_nat[:], a.ap()[m * P : (m + 1) * P, :])
                scratch = scr_pool.tile([P, K], dt.float32, tag="scratch", name="scratch")
                me.tensor_tensor(scratch[:], a_nat[:], w0b[:], Alu.mult)
                return scratch

            def cond_finish(m, scratch):
                c0 = d_pool.tile([P, 1], dt.float32, tag=f"c0_{m}", name=f"c0_{m}")
                nc.vector.tensor_reduce(
                    c0[:], scratch[:], mybir.AxisListType.X, Alu.add
                )
                g = d_pool.tile([P, 1], dt.float32, tag=f"g_{m}", name=f"g_{m}")
                nc.vector.tensor_scalar(
                    g[:], c0[:], b128[:, 0:1], 0.5, Alu.add, Alu.is_gt
                )
                d = d_pool.tile([P, 1], dt.float32, tag=f"d_{m}", name=f"d_{m}")
                nc.vector.tensor_scalar(d[:], g[:], 2.0, -1.0, Alu.mult, Alu.add)
                return d

            # conds 0/1: a_nat on sync right after its W half (guaranteed
            # early), multiply on DVE too (data arrives before pair 0 ends)
            conds = {}
            scrs = {}
            scrs[0] = cond_start(0, nc.sync, nc.vector)
            scrs[1] = cond_start(1, nc.sync, nc.vector)
            conds[0] = cond_finish(0, scrs[0])
            conds[1] = cond_finish(1, scrs[1])

            # remaining atp tiles after the a_nat0/1 loads
            load_atp(1, nc.scalar)
            load_atp(2, nc.sync)
            load_atp(3, nc.sync)

            def epilogue(psum, d, m, nq):
                out_sb = out_pool.tile([P, N_SUB], dt.float32, tag="out_sb", name="out_sb")
                nc.vector.scalar_tensor_tensor(
                    out_sb[:],
                    psum[:],
                    d[:],
                    b128[:, nq * N_SUB : (nq + 1) * N_SUB],
                    Alu.add,
                    Alu.add,
                )
                nc.sync.dma_start(
                    out.ap()[m * P : (m + 1) * P, nq * N_SUB : (nq + 1) * N_SUB],
                    out_sb[:],
                )

            for pr in range(PAIRS):
                m0 = 2 * pr
                if pr >= 1:
                    # DVE part for this pair's conds (mults already issued
                    # on gpsimd one pair ago, data long since arrived)
                    conds[m0] = cond_finish(m0, scrs[m0])
                    conds[m0 + 1] = cond_finish(m0 + 1, scrs[m0 + 1])
                if pr + 1 < PAIRS:
                    # kick off next pair's a_nat DMAs (scalar queue, behind
                    # atp1) and multiplies (gpsimd)
                    scrs[m0 + 2] = cond_start(m0 + 2, nc.scalar, nc.gpsimd)
                    scrs[m0 + 3] = cond_start(m0 + 3, nc.scalar, nc.gpsimd)
                ds = [conds[m0], conds[m0 + 1]]
                atp = atps[pr]

                if pr == 0:
                    # ko-group outer in DMA arrival order, streaming right
                    # behind the W group DMAs
                    psums = [
                        [
                            psum_pool.tile(
                                [P, N_SUB], dt.float32, name=f"ps_{mi}_{nq}", tag="psum"
                            )
                            for nq in range(NQ)
                        ]
                        for mi in range(2)
                    ]
                    for gi, g4 in enumerate(CONSUME0):
                        for kk in range(KGS):
                            ko = g4 * KGS + kk
                            for mi in range(2):
                                for nq in range(NQ):
                                    nc.tensor.matmul(
                                        psums[mi][nq][:],
                                        atp[:, ko, mi * P : (mi + 1) * P],
                                        wgs[g4][:, kk, nq * N_SUB : (nq + 1) * N_SUB],
                                        start=(gi == 0 and kk == 0),
                                        stop=(gi == KG - 1 and kk == KGS - 1),
                                    )
                    for mi in range(2):
                        for nq in range(NQ):
                            epilogue(psums[mi][nq], ds[mi], m0 + mi, nq)
                else:
                    # W fully resident: ko-inner per psum so each tile closes
                    # early and its epilogue+store overlaps remaining matmuls
                    for mi in range(2):
                        for nq in range(NQ):
                            psum = psum_pool.tile(
                                [P, N_SUB], dt.float32, name="ps", tag="psum"
                            )
                            for ko in range(KO):
                                nc.tensor.matmul(
                                    psum[:],
                                    atp[:, ko, mi * P : (mi + 1) * P],
                                    wgs[ko // KGS][:, ko % KGS, nq * N_SUB : (nq + 1) * N_SUB],
                                    start=(ko == 0),
                                    stop=(ko == KO - 1),
                                )
                            epilogue(psum, ds[mi], m0 + mi, nq)

    nc.compile()
    return nc


_NC = None


def _get_nc():
    global _NC
    if _NC is None:
        _NC = _build()
    return _NC


def build_in_maps(a, w, b):
    """Host-side prep: shard A, pre-transpose/block to bf16, cast W to bf16."""
    a = np.ascontiguousarray(a, dtype=np.float32)
    w = np.ascontiguousarray(w, dtype=np.float32)
    b = np.ascontiguousarray(b, dtype=np.float32)
    # w_blk[g, kp, kk, n] = W[(g*KGS+kk)*P + kp, n], bf16
    w_blk = np.ascontiguousarray(
        np.transpose(w.astype(BF16).reshape(KG, KGS, P, N), (0, 2, 1, 3))
    )
    w0 = np.ascontiguousarray(w[:, 0].reshape(1, K))
    in_maps = []
    for i in range(N_CORES):
        a_sh = a[i * M_SHARD : (i + 1) * M_SHARD]
        # at[pr, kp, ko, mm] = a_sh[pr*256+mm, ko*128+kp]
        at = np.transpose(
            a_sh.reshape(PAIRS, MP, KO, P), (0, 3, 2, 1)
        ).astype(BF16)
        in_maps.append(
            {"inputs": a_sh, "at": at, "w": w_blk, "b": b, "w0": w0}
        )
    return in_maps


def kernel(**inputs: np.ndarray) -> np.ndarray:
    a = inputs["inputs"]
    assert a.shape == (B_FULL, K), a.shape
    nc = _get_nc()
    in_maps = build_in_maps(a, inputs["w"], inputs["b"])
    res = run_bass_kernel_spmd(nc, in_maps, core_ids=list(range(N_CORES)))
    return np.concatenate([res.results[i]["out"] for i in range(N_CORES)], axis=0)


# revision 23
# speedup vs baseline: 1.0756x; 1.0378x over previous
"""TRN2 Bass kernel for nn_LinearBinary: out = (A @ W + b) +/- 1 per-row.

    A: [8192, 2048] f32, W: [2048, 2048] f32, b: [2048] f32
    C = A @ W + b;  cond = C[:, :1] > 0.5;  out = where(cond, C+1, C-1)

Sharding: data-parallel over the 8192-row batch across 8 NeuronCores
(1024 rows/core); W and b replicated. SPMD - one program, per-core shards
via in_maps.

Per-core kernel (v3 — PE-minimal, DMA-tuned):
  - W shipped as bf16 (8MB) and kept fully resident in SBUF as 4
    ko-groups, streamed on the sync (SP) HWDGE queue with 4KB packets.
  - A^T shipped pre-transposed bf16 from the host, blocked per m-pair as
    [pair, kp, ko, m] so each partition line is one contiguous 8KB run:
    DMA arbitration is packet-round-robin, so packet size IS bandwidth
    share — 512B A^T packets get starved 8:1 by 4KB W packets.
  - PE does ZERO transposes — only the 512 bf16 matmuls (1 cyc/row,
    262144 cycles ~= 109us at 2.4GHz).
  - m-tiles in PAIRS across all 8 PSUM banks. Pair 0 runs ko-group-outer
    so its matmuls stream right behind the W group DMAs; pairs 1-3 run
    ko-inner per psum so each [128,512] tile closes early and its
    epilogue+store overlaps the remaining matmuls (kills the end-of-pair
    store burst and the final tail).
  - The row condition needs exact fp32 C[:, 0] (min |C0-0.5| margin on
    this data is ~4.4e-4; bf16 A would flip rows): computed on
    gpsimd (mult) + DVE (reduce) from the natural-layout fp32 A.
  - Epilogue fuses (psum + (-+1)) + b in one scalar_tensor_tensor per
    [128, 512] tile; stores go on the sync queue (idle once W landed).
"""

import sys

for _p in ("/opt/trn_rl_repo", "/root/.axon_site/_ro/trn_rl_repo"):
    if _p not in sys.path:
        sys.path.append(_p)

import ml_dtypes
import numpy as np

import concourse.bacc as bacc
import concourse.mybir as mybir
import concourse.tile as tile
from concourse.bass_utils import run_bass_kernel_spmd

dt = mybir.dt
Alu = mybir.AluOpType
BF16 = np.dtype(ml_dtypes.bfloat16)

P = 128
K = 2048
N = 2048
B_FULL = 8192
N_CORES = 8
M_SHARD = B_FULL // N_CORES  # 1024 rows per core
M_TILES = M_SHARD // P  # 8
KO = K // P  # 16
KG = 8  # W ko-groups (DMA granularity)
KGS = KO // KG  # 2 ko per group
NQ = 4  # PSUM n-chunks
N_SUB = N // NQ  # 512
PAIRS = M_TILES // 2  # m-tiles processed 2 at a time (8 PSUM banks)
MP = 2 * P  # rows per pair


def _build():
    nc = bacc.Bacc("TRN2", target_bir_lowering=False, debug=False, num_devices=N_CORES)

    a = nc.dram_tensor("inputs", [M_SHARD, K], dt.float32, kind="ExternalInput")
    # A^T, host-blocked per m-pair: at[pr, kp, ko, mm] = A[pr*256+mm, ko*128+kp]
    at = nc.dram_tensor("at", [PAIRS, P, KO, MP], dt.bfloat16, kind="ExternalInput")
    # W host-blocked per ko-group: w[g, kp, kk, n] = W[(g*KGS+kk)*128+kp, n]
    # so each partition line is one contiguous 8KB run (big DMA packets win
    # the packet-round-robin arbitration).
    w = nc.dram_tensor("w", [KG, P, KGS, N], dt.bfloat16, kind="ExternalInput")
    b = nc.dram_tensor("b", [N], dt.float32, kind="ExternalInput")
    # W[:, 0] pre-sliced on host: a strided 4-byte column-gather DMA is fatal
    # on HW (NRT_EXEC_UNIT_UNRECOVERABLE), so ship the 8KB row directly.
    w0 = nc.dram_tensor("w0", [1, K], dt.float32, kind="ExternalInput")
    out = nc.dram_tensor("out", [M_SHARD, N], dt.float32, kind="ExternalOutput")

    with tile.TileContext(nc) as tc:
        with (
            tc.tile_pool(name="consts", bufs=1) as consts,
            tc.tile_pool(name="wg", bufs=1) as wg_pool,
            tc.tile_pool(name="atp", bufs=1) as at_pool,
            tc.tile_pool(name="anat", bufs=2) as anat_pool,
            tc.tile_pool(name="scr", bufs=2) as scr_pool,
            tc.tile_pool(name="dsm", bufs=1) as d_pool,
            tc.tile_pool(name="outs", bufs=4) as out_pool,
            tc.tile_pool(name="psc", bufs=8, space="PSUM") as psum_pool,
        ):
            # b and W[:, 0] broadcast to all partitions (scalar queue: tiny)
            b_row = consts.tile([1, N], dt.float32, tag="b_row")
            nc.scalar.dma_start(b_row[:], b.ap().unsqueeze(0))
            b128 = consts.tile([P, N], dt.float32, tag="b128")
            nc.gpsimd.partition_broadcast(b128[:], b_row[:])
            w0_row = consts.tile([1, K], dt.float32, tag="w0_row")
            nc.sync.dma_start(w0_row[:], w0.ap())
            w0b = consts.tile([P, K], dt.float32, tag="w0b")
            nc.gpsimd.partition_broadcast(w0b[:], w0_row[:])

            # warm-up: dummy matmuls on a zeroed bf16 tile while the first
            # DMAs land, so the PE p-state ramp happens off the critical path
            warm = consts.tile([P, P], dt.bfloat16, tag="warm")
            nc.vector.memset(warm[:], 0.0)
            ps_w = psum_pool.tile([P, N_SUB], dt.float32, tag="psum", name="ps_w")
            for _ in range(8):
                nc.tensor.matmul(
                    ps_w[:, :P], warm[:], warm[:], start=True, stop=True
                )

            # W resident, bf16, 8 ko-groups of 1MB. The bandwidth cap is
            # per-HWDGE-queue (~180GB/s each), so split W across BOTH
            # HWDGE queues; atp0 and g0 (which together gate the first
            # matmul) go FIRST on opposite queues. Pair 0 consumes groups
            # in merged arrival order CONSUME0 below.
            atps = []
            for pr in range(PAIRS):
                atp = at_pool.tile([P, KO, MP], dt.bfloat16, tag=f"atp{pr}", name=f"atp{pr}")
                atps.append(atp)

            def load_atp(pr, eng):
                eng.dma_start(atps[pr][:], at.ap()[pr])

            load_atp(0, nc.sync)

            wgs = [None] * KG
            issue = [(0, nc.scalar), (2, nc.sync), (1, nc.scalar), (3, nc.sync),
                     (4, nc.scalar), (6, nc.sync), (5, nc.scalar), (7, nc.scalar)]
            for g, eng in issue:
                wg = wg_pool.tile([P, KGS, N], dt.bfloat16, tag=f"wg{g}", name=f"wg{g}")
                eng.dma_start(wg[:], w.ap()[g])
                wgs[g] = wg
            # arrival: scalar g0@~14,g1@~19,g4@~25,g5@~30,g7@~36;
            # sync atp0@~14,g2@~19,g3@~25,g6@~30
            CONSUME0 = [0, 2, 1, 3, 4, 6, 5, 7]

            # exact fp32 condition: d = (A[m-tile] @ w0 + b0 > 0.5) ? +1 : -1
            # All on DVE, issued inline; PSUM ping-pong below keeps these
            # off every critical path.
            def cond_m(m, dma_eng):
                a_nat = anat_pool.tile([P, K], dt.float32, tag="a_nat", name="a_nat")
                dma_eng.dma_start(a_nat[:], a.ap()[m * P : (m + 1) * P, :])
                scratch = scr_pool.tile([P, K], dt.float32, tag="scratch", name="scratch")
                nc.vector.tensor_tensor(scratch[:], a_nat[:], w0b[:], Alu.mult)
                c0 = d_pool.tile([P, 1], dt.float32, tag=f"c0_{m}", name=f"c0_{m}")
                nc.vector.tensor_reduce(
                    c0[:], scratch[:], mybir.AxisListType.X, Alu.add
                )
                g = d_pool.tile([P, 1], dt.float32, tag=f"g_{m}", name=f"g_{m}")
                nc.vector.tensor_scalar(
                    g[:], c0[:], b128[:, 0:1], 0.5, Alu.add, Alu.is_gt
                )
                d = d_pool.tile([P, 1], dt.float32, tag=f"d_{m}", name=f"d_{m}")
                nc.vector.tensor_scalar(d[:], g[:], 2.0, -1.0, Alu.mult, Alu.add)
                return d

            # conds 0/1: a_nat on sync right after its W half (guaranteed
            # early, so d0/d1 exist before pair 0's psums close)
            conds = {}
            conds[0] = cond_m(0, nc.sync)
            conds[1] = cond_m(1, nc.sync)

            # remaining atp tiles after the a_nat0/1 loads
            load_atp(1, nc.scalar)
            load_atp(2, nc.sync)
            load_atp(3, nc.sync)

            def epilogue(psum, d, m, nq):
                out_sb = out_pool.tile([P, N_SUB], dt.float32, tag="out_sb", name="out_sb")
                nc.vector.scalar_tensor_tensor(
                    out_sb[:],
                    psum[:],
                    d[:],
                    b128[:, nq * N_SUB : (nq + 1) * N_SUB],
                    Alu.add,
                    Alu.add,
                )
                nc.sync.dma_start(
                    out.ap()[m * P : (m + 1) * P, nq * N_SUB : (nq + 1) * N_SUB],
                    out_sb[:],
                )

            # pair 0: ko-group outer in DMA arrival order, streaming right
            # behind the W group DMAs, both m-tiles across all 8 PSUM banks
            psums = [
                [
                    psum_pool.tile(
                        [P, N_SUB], dt.float32, name=f"ps_{mi}_{nq}", tag="psum"
                    )
                    for nq in range(NQ)
                ]
                for mi in range(2)
            ]
            for gi, g4 in enumerate(CONSUME0):
                for kk in range(KGS):
                    ko = g4 * KGS + kk
                    for mi in range(2):
                        for nq in range(NQ):
                            nc.tensor.matmul(
                                psums[mi][nq][:],
                                atps[0][:, ko, mi * P : (mi + 1) * P],
                                wgs[g4][:, kk, nq * N_SUB : (nq + 1) * N_SUB],
                                start=(gi == 0 and kk == 0),
                                stop=(gi == KG - 1 and kk == KGS - 1),
                            )
            for mi in range(2):
                for nq in range(NQ):
                    epilogue(psums[mi][nq], conds[mi], mi, nq)

            # m-tiles 2..7 SINGLY: 4 psums each, so consecutive m-tiles
            # ping-pong between bank halves and an epilogue stt has a full
            # m-tile period (~14us) before its bank is needed again — late
            # conds can never stall the PE.
            for m in range(2, M_TILES):
                d = cond_m(m, nc.scalar)
                atp = atps[m // 2]
                mi = m % 2
                for nq in range(NQ):
                    psum = psum_pool.tile(
                        [P, N_SUB], dt.float32, name="ps", tag="psum"
                    )
                    for ko in range(KO):
                        nc.tensor.matmul(
                            psum[:],
                            atp[:, ko, mi * P : (mi + 1) * P],
                            wgs[ko // KGS][:, ko % KGS, nq * N_SUB : (nq + 1) * N_SUB],
                            start=(ko == 0),
                            stop=(ko == KO - 1),
                        )
                    epilogue(psum, d, m, nq)

    nc.compile()
    return nc


_NC = None


def _get_nc():
    global _NC
    if _NC is None:
        _NC = _build()
    return _NC


def build_in_maps(a, w, b):
    """Host-side prep: shard A, pre-transpose/block to bf16, cast W to bf16."""
    a = np.ascontiguousarray(a, dtype=np.float32)
    w = np.ascontiguousarray(w, dtype=np.float32)
    b = np.ascontiguousarray(b, dtype=np.float32)
    # w_blk[g, kp, kk, n] = W[(g*KGS+kk)*P + kp, n], bf16
    w_blk = np.ascontiguousarray(
        np.transpose(w.astype(BF16).reshape(KG, KGS, P, N), (0, 2, 1, 3))
    )
    w0 = np.ascontiguousarray(w[:, 0].reshape(1, K))
    in_maps = []
    for i in range(N_CORES):
        a_sh = a[i * M_SHARD : (i + 1) * M_SHARD]
        # at[pr, kp, ko, mm] = a_sh[pr*256+mm, ko*128+kp]
        at = np.transpose(
            a_sh.reshape(PAIRS, MP, KO, P), (0, 3, 2, 1)
        ).astype(BF16)
        in_maps.append(
            {"inputs": a_sh, "at": at, "w": w_blk, "b": b, "w0": w0}
        )
    return in_maps


def kernel(**inputs: np.ndarray) -> np.ndarray:
    a = inputs["inputs"]
    assert a.shape == (B_FULL, K), a.shape
    nc = _get_nc()
    in_maps = build_in_maps(a, inputs["w"], inputs["b"])
    res = run_bass_kernel_spmd(nc, in_maps, core_ids=list(range(N_CORES)))
    return np.concatenate([res.results[i]["out"] for i in range(N_CORES)], axis=0)


# revision 30
# speedup vs baseline: 1.1190x; 1.0404x over previous
"""TRN2 Bass kernel for nn_LinearBinary: out = (A @ W + b) +/- 1 per-row.

    A: [8192, 2048] f32, W: [2048, 2048] f32, b: [2048] f32
    C = A @ W + b;  cond = C[:, :1] > 0.5;  out = where(cond, C+1, C-1)

Sharding: data-parallel over the 8192-row batch across 8 NeuronCores
(1024 rows/core); W and b replicated. SPMD - one program, per-core shards
via in_maps.

Per-core kernel (v3 — PE-minimal, DMA-tuned):
  - W shipped as bf16 (8MB) and kept fully resident in SBUF as 4
    ko-groups, streamed on the sync (SP) HWDGE queue with 4KB packets.
  - A^T shipped pre-transposed bf16 from the host, blocked per m-pair as
    [pair, kp, ko, m] so each partition line is one contiguous 8KB run:
    DMA arbitration is packet-round-robin, so packet size IS bandwidth
    share — 512B A^T packets get starved 8:1 by 4KB W packets.
  - PE does ZERO transposes — only the 512 bf16 matmuls (1 cyc/row,
    262144 cycles ~= 109us at 2.4GHz).
  - m-tiles in PAIRS across all 8 PSUM banks. Pair 0 runs ko-group-outer
    so its matmuls stream right behind the W group DMAs; pairs 1-3 run
    ko-inner per psum so each [128,512] tile closes early and its
    epilogue+store overlaps the remaining matmuls (kills the end-of-pair
    store burst and the final tail).
  - The row condition needs exact fp32 C[:, 0] (min |C0-0.5| margin on
    this data is ~4.4e-4; bf16 A would flip rows): computed on
    gpsimd (mult) + DVE (reduce) from the natural-layout fp32 A.
  - Epilogue fuses (psum + (-+1)) + b in one scalar_tensor_tensor per
    [128, 512] tile; stores go on the sync queue (idle once W landed).
"""

import sys

for _p in ("/opt/trn_rl_repo", "/root/.axon_site/_ro/trn_rl_repo"):
    if _p not in sys.path:
        sys.path.append(_p)

import ml_dtypes
import numpy as np

import concourse.bacc as bacc
import concourse.mybir as mybir
import concourse.tile as tile
from concourse.bass_utils import run_bass_kernel_spmd

dt = mybir.dt
Alu = mybir.AluOpType
BF16 = np.dtype(ml_dtypes.bfloat16)

P = 128
K = 2048
N = 2048
B_FULL = 8192
N_CORES = 8
M_SHARD = B_FULL // N_CORES  # 1024 rows per core
M_TILES = M_SHARD // P  # 8
KO = K // P  # 16
KG = 8  # W ko-groups (DMA granularity)
KGS = KO // KG  # 2 ko per group
NQ = 4  # PSUM n-chunks
N_SUB = N // NQ  # 512
PAIRS = M_TILES // 2  # m-tiles processed 2 at a time (8 PSUM banks)
MP = 2 * P  # rows per pair


def _build():
    nc = bacc.Bacc("TRN2", target_bir_lowering=False, debug=False, num_devices=N_CORES)

    a = nc.dram_tensor("inputs", [M_SHARD, K], dt.float32, kind="ExternalInput")
    # A^T, host-blocked per m-tile: at[t, kp, ko, mm] = A[t*128+mm, ko*128+kp]
    at = nc.dram_tensor("at", [M_TILES, P, KO, P], dt.bfloat16, kind="ExternalInput")
    # W host-blocked per ko-group: w[g, kp, kk, n] = W[(g*KGS+kk)*128+kp, n]
    # so each partition line is one contiguous 8KB run (big DMA packets win
    # the packet-round-robin arbitration).
    w = nc.dram_tensor("w", [KG, P, KGS, N], dt.bfloat16, kind="ExternalInput")
    b = nc.dram_tensor("b", [N], dt.float32, kind="ExternalInput")
    # W[:, 0] pre-sliced on host: a strided 4-byte column-gather DMA is fatal
    # on HW (NRT_EXEC_UNIT_UNRECOVERABLE), so ship the 8KB row directly.
    w0 = nc.dram_tensor("w0", [1, K], dt.float32, kind="ExternalInput")
    out = nc.dram_tensor("out", [M_SHARD, N], dt.float32, kind="ExternalOutput")

    with tile.TileContext(nc) as tc:
        with (
            tc.tile_pool(name="consts", bufs=1) as consts,
            tc.tile_pool(name="wg", bufs=1) as wg_pool,
            tc.tile_pool(name="atp", bufs=1) as at_pool,
            tc.tile_pool(name="anat", bufs=1) as anat_pool,
            tc.tile_pool(name="scr", bufs=2) as scr_pool,
            tc.tile_pool(name="dsm", bufs=1) as d_pool,
            tc.tile_pool(name="outs", bufs=4) as out_pool,
            tc.tile_pool(name="psc", bufs=8, space="PSUM") as psum_pool,
        ):
            # b and W[:, 0] broadcast to all partitions (scalar queue: tiny)
            b_row = consts.tile([1, N], dt.float32, tag="b_row")
            nc.scalar.dma_start(b_row[:], b.ap().unsqueeze(0))
            b128 = consts.tile([P, N], dt.float32, tag="b128")
            nc.gpsimd.partition_broadcast(b128[:], b_row[:])
            w0_row = consts.tile([1, K], dt.float32, tag="w0_row")
            nc.sync.dma_start(w0_row[:], w0.ap())
            w0b = consts.tile([P, K], dt.float32, tag="w0b")
            nc.gpsimd.partition_broadcast(w0b[:], w0_row[:])

            # warm-up: dummy matmuls on a zeroed bf16 tile while the first
            # DMAs land, so the PE p-state ramp happens off the critical path
            warm = consts.tile([P, N_SUB], dt.bfloat16, tag="warm")
            nc.vector.memset(warm[:], 0.0)
            ps_w = psum_pool.tile([P, N_SUB], dt.float32, tag="psum", name="ps_w")
            for _ in range(8):
                nc.tensor.matmul(
                    ps_w[:], warm[:, :P], warm[:], start=True, stop=True
                )

            # W resident, bf16, 8 ko-groups of 1MB. The bandwidth cap is
            # per-HWDGE-queue (~180GB/s each), so split traffic across BOTH
            # HWDGE queues; atm0 and g0 (which together gate the first
            # matmul) go FIRST on opposite queues, g0 in two 0.5MB ko-halves
            # so matmuls start ~3us earlier. Pair 0 consumes groups in
            # merged arrival order CONSUME0 below.
            atms = []
            for t in range(M_TILES):
                atm = at_pool.tile([P, KO, P], dt.bfloat16, tag=f"atm{t}", name=f"atm{t}")
                atms.append(atm)

            def load_atm(t, eng):
                eng.dma_start(atms[t][:], at.ap()[t])

            load_atm(0, nc.sync)
            load_atm(1, nc.sync)

            wgs = [None] * KG
            for g in range(KG):
                wgs[g] = wg_pool.tile(
                    [P, KGS, N], dt.bfloat16, tag=f"wg{g}", name=f"wg{g}"
                )
            # scalar: g0 (2 halves), g1, g4, g5, g7; sync: g2, g3, (g6 later)
            for kk in range(KGS):
                nc.scalar.dma_start(
                    wgs[0][:, kk : kk + 1, :], w.ap()[0, :, kk : kk + 1, :]
                )
            nc.sync.dma_start(wgs[2][:], w.ap()[2])
            nc.scalar.dma_start(wgs[1][:], w.ap()[1])
            nc.sync.dma_start(wgs[3][:], w.ap()[3])
            nc.scalar.dma_start(wgs[4][:], w.ap()[4])
            nc.scalar.dma_start(wgs[5][:], w.ap()[5])
            nc.scalar.dma_start(wgs[7][:], w.ap()[7])
            CONSUME0 = [0, 1, 2, 3, 4, 5, 7, 6]

            # exact fp32 condition: d = (A[m-tile] @ w0 + b0 > 0.5) ? +1 : -1
            # All on DVE, issued inline; PSUM ping-pong below keeps these
            # off every critical path.
            def cond_m(m, dma_eng):
                # alternating tags -> distinct semaphores, so cond m's mult
                # never waits on a later a_nat DMA via sem coarsening
                a_nat = anat_pool.tile(
                    [P, K], dt.float32, tag=f"a_nat_{m % 2}", name="a_nat"
                )
                dma_eng.dma_start(a_nat[:], a.ap()[m * P : (m + 1) * P, :])
                scratch = scr_pool.tile([P, K], dt.float32, tag="scratch", name="scratch")
                nc.vector.tensor_tensor(scratch[:], a_nat[:], w0b[:], Alu.mult)
                c0 = d_pool.tile([P, 1], dt.float32, tag=f"c0_{m}", name=f"c0_{m}")
                nc.vector.tensor_reduce(
                    c0[:], scratch[:], mybir.AxisListType.X, Alu.add
                )
                g = d_pool.tile([P, 1], dt.float32, tag=f"g_{m}", name=f"g_{m}")
                nc.vector.tensor_scalar(
                    g[:], c0[:], b128[:, 0:1], 0.5, Alu.add, Alu.is_gt
                )
                d = d_pool.tile([P, 1], dt.float32, tag=f"d_{m}", name=f"d_{m}")
                nc.vector.tensor_scalar(d[:], g[:], 2.0, -1.0, Alu.mult, Alu.add)
                return d

            # conds 0/1 on sync interleaved with the rest of sync's payload
            # (anat0 before g6 so d0 exists before pair 0's psums close);
            # remaining atm tiles spread across both queues
            conds = {}
            conds[0] = cond_m(0, nc.sync)
            load_atm(2, nc.sync)
            nc.sync.dma_start(wgs[6][:], w.ap()[6])
            conds[1] = cond_m(1, nc.sync)
            load_atm(3, nc.sync)
            load_atm(4, nc.sync)
            load_atm(5, nc.sync)
            load_atm(6, nc.scalar)
            load_atm(7, nc.scalar)

            def epilogue(psum, d, m, nq):
                out_sb = out_pool.tile([P, N_SUB], dt.float32, tag="out_sb", name="out_sb")
                nc.vector.scalar_tensor_tensor(
                    out_sb[:],
                    psum[:],
                    d[:],
                    b128[:, nq * N_SUB : (nq + 1) * N_SUB],
                    Alu.add,
                    Alu.add,
                )
                nc.sync.dma_start(
                    out.ap()[m * P : (m + 1) * P, nq * N_SUB : (nq + 1) * N_SUB],
                    out_sb[:],
                )

            # pair 0: ko-group outer in DMA arrival order, streaming right
            # behind the W group DMAs, both m-tiles across all 8 PSUM banks
            psums = [
                [
                    psum_pool.tile(
                        [P, N_SUB], dt.float32, name=f"ps_{mi}_{nq}", tag="psum"
                    )
                    for nq in range(NQ)
                ]
                for mi in range(2)
            ]
            for gi, g4 in enumerate(CONSUME0):
                for kk in range(KGS):
                    ko = g4 * KGS + kk
                    for mi in range(2):
                        for nq in range(NQ):
                            nc.tensor.matmul(
                                psums[mi][nq][:],
                                atms[mi][:, ko, :],
                                wgs[g4][:, kk, nq * N_SUB : (nq + 1) * N_SUB],
                                start=(gi == 0 and kk == 0),
                                stop=(gi == KG - 1 and kk == KGS - 1),
                            )
            for mi in range(2):
                for nq in range(NQ):
                    epilogue(psums[mi][nq], conds[mi], mi, nq)

            # m-tiles 2..7 SINGLY: 4 psums each, so consecutive m-tiles
            # ping-pong between bank halves and an epilogue stt has a full
            # m-tile period (~14us) before its bank is needed again — late
            # conds can never stall the PE. Their cond chains are pushed to
            # LOW scheduler priority so the Tile scheduler slots them after
            # earlier epilogues in the in-order DVE queue.
            for m in range(2, M_TILES):
                with tc.high_priority(offset=-1_000_000):
                    d = cond_m(m, nc.scalar)
                for nq in range(NQ):
                    psum = psum_pool.tile(
                        [P, N_SUB], dt.float32, name="ps", tag="psum"
                    )
                    for ko in range(KO):
                        nc.tensor.matmul(
                            psum[:],
                            atms[m][:, ko, :],
                            wgs[ko // KGS][:, ko % KGS, nq * N_SUB : (nq + 1) * N_SUB],
                            start=(ko == 0),
                            stop=(ko == KO - 1),
                        )
                    epilogue(psum, d, m, nq)

    nc.compile()
    return nc


_NC = None


def _get_nc():
    global _NC
    if _NC is None:
        _NC = _build()
    return _NC


def build_in_maps(a, w, b):
    """Host-side prep: shard A, pre-transpose/block to bf16, cast W to bf16."""
    a = np.ascontiguousarray(a, dtype=np.float32)
    w = np.ascontiguousarray(w, dtype=np.float32)
    b = np.ascontiguousarray(b, dtype=np.float32)
    # w_blk[g, kp, kk, n] = W[(g*KGS+kk)*P + kp, n], bf16
    w_blk = np.ascontiguousarray(
        np.transpose(w.astype(BF16).reshape(KG, KGS, P, N), (0, 2, 1, 3))
    )
    w0 = np.ascontiguousarray(w[:, 0].reshape(1, K))
    in_maps = []
    for i in range(N_CORES):
        a_sh = a[i * M_SHARD : (i + 1) * M_SHARD]
        # at[t, kp, ko, mm] = a_sh[t*128+mm, ko*128+kp]
        at = np.transpose(
            a_sh.reshape(M_TILES, P, KO, P), (0, 3, 2, 1)
        ).astype(BF16)
        in_maps.append(
            {"inputs": a_sh, "at": at, "w": w_blk, "b": b, "w0": w0}
        )
    return in_maps


def kernel(**inputs: np.ndarray) -> np.ndarray:
    a = inputs["inputs"]
    assert a.shape == (B_FULL, K), a.shape
    nc = _get_nc()
    in_maps = build_in_maps(a, inputs["w"], inputs["b"])
    res = run_bass_kernel_spmd(nc, in_maps, core_ids=list(range(N_CORES)))
    return np.concatenate([res.results[i]["out"] for i in range(N_CORES)], axis=0)


# revision 38
# speedup vs baseline: 1.1588x; 1.0356x over previous
"""TRN2 Bass kernel for nn_LinearBinary: out = (A @ W + b) +/- 1 per-row.

    A: [8192, 2048] f32, W: [2048, 2048] f32, b: [2048] f32
    C = A @ W + b;  cond = C[:, :1] > 0.5;  out = where(cond, C+1, C-1)

Sharding: data-parallel over the 8192-row batch across 8 NeuronCores
(1024 rows/core); W and b replicated. SPMD - one program, per-core shards
via in_maps.

Per-core kernel (v3 — PE-minimal, DMA-tuned):
  - W shipped as bf16 (8MB) and kept fully resident in SBUF as 4
    ko-groups, streamed on the sync (SP) HWDGE queue with 4KB packets.
  - A^T shipped pre-transposed bf16 from the host, blocked per m-pair as
    [pair, kp, ko, m] so each partition line is one contiguous 8KB run:
    DMA arbitration is packet-round-robin, so packet size IS bandwidth
    share — 512B A^T packets get starved 8:1 by 4KB W packets.
  - PE does ZERO transposes — only the 512 bf16 matmuls (1 cyc/row,
    262144 cycles ~= 109us at 2.4GHz).
  - m-tiles in PAIRS across all 8 PSUM banks. Pair 0 runs ko-group-outer
    so its matmuls stream right behind the W group DMAs; pairs 1-3 run
    ko-inner per psum so each [128,512] tile closes early and its
    epilogue+store overlaps the remaining matmuls (kills the end-of-pair
    store burst and the final tail).
  - The row condition needs exact fp32 C[:, 0] (min |C0-0.5| margin on
    this data is ~4.4e-4; bf16 A would flip rows): computed on
    gpsimd (mult) + DVE (reduce) from the natural-layout fp32 A.
  - Epilogue fuses (psum + (-+1)) + b in one scalar_tensor_tensor per
    [128, 512] tile; stores go on the sync queue (idle once W landed).
"""

import sys

for _p in ("/opt/trn_rl_repo", "/root/.axon_site/_ro/trn_rl_repo"):
    if _p not in sys.path:
        sys.path.append(_p)

import ml_dtypes
import numpy as np

import concourse.bacc as bacc
import concourse.mybir as mybir
import concourse.tile as tile
from concourse.bass_utils import run_bass_kernel_spmd
from concourse.tile import add_dep_helper

dt = mybir.dt
Alu = mybir.AluOpType
BF16 = np.dtype(ml_dtypes.bfloat16)

P = 128
K = 2048
N = 2048
B_FULL = 8192
N_CORES = 8
M_SHARD = B_FULL // N_CORES  # 1024 rows per core
M_TILES = M_SHARD // P  # 8
KO = K // P  # 16
KG = 8  # W ko-groups (DMA granularity)
KGS = KO // KG  # 2 ko per group
NQ = 4  # PSUM n-chunks
N_SUB = N // NQ  # 512
PAIRS = M_TILES // 2  # m-tiles processed 2 at a time (8 PSUM banks)
MP = 2 * P  # rows per pair


def _build():
    nc = bacc.Bacc("TRN2", target_bir_lowering=False, debug=False, num_devices=N_CORES)

    a = nc.dram_tensor("inputs", [M_SHARD, K], dt.float32, kind="ExternalInput")
    # A^T, host-blocked per m-tile: at[t, kp, ko, mm] = A[t*128+mm, ko*128+kp]
    at = nc.dram_tensor("at", [M_TILES, P, KO, P], dt.bfloat16, kind="ExternalInput")
    # W host-blocked per ko-group: w[g, kp, kk, n] = W[(g*KGS+kk)*128+kp, n]
    # so each partition line is one contiguous 8KB run (big DMA packets win
    # the packet-round-robin arbitration).
    w = nc.dram_tensor("w", [KG, P, KGS, N], dt.bfloat16, kind="ExternalInput")
    b = nc.dram_tensor("b", [N], dt.float32, kind="ExternalInput")
    # W[:, 0] pre-sliced on host: a strided 4-byte column-gather DMA is fatal
    # on HW (NRT_EXEC_UNIT_UNRECOVERABLE), so ship the 8KB row directly.
    w0 = nc.dram_tensor("w0", [1, K], dt.float32, kind="ExternalInput")
    out = nc.dram_tensor("out", [M_SHARD, N], dt.float32, kind="ExternalOutput")

    with tile.TileContext(nc) as tc:
        with (
            tc.tile_pool(name="consts", bufs=1) as consts,
            tc.tile_pool(name="wg", bufs=1) as wg_pool,
            tc.tile_pool(name="atp", bufs=1) as at_pool,
            tc.tile_pool(name="anat", bufs=1) as anat_pool,
            tc.tile_pool(name="scr", bufs=2) as scr_pool,
            tc.tile_pool(name="dsm", bufs=1) as d_pool,
            tc.tile_pool(name="outs", bufs=4) as out_pool,
            tc.tile_pool(name="psc", bufs=8, space="PSUM") as psum_pool,
        ):
            # b and W[:, 0] broadcast to all partitions (sync queue: tiny)
            b_row = consts.tile([1, N], dt.float32, tag="b_row")
            nc.sync.dma_start(b_row[:], b.ap().unsqueeze(0))
            b128 = consts.tile([P, N], dt.float32, tag="b128")
            nc.gpsimd.partition_broadcast(b128[:], b_row[:])
            w0_row = consts.tile([1, K], dt.float32, tag="w0_row")
            nc.sync.dma_start(w0_row[:], w0.ap())
            w0b = consts.tile([P, K], dt.float32, tag="w0b")
            nc.gpsimd.partition_broadcast(w0b[:], w0_row[:])

            # warm-up: dummy matmuls on a zeroed bf16 tile while the first
            # DMAs land, so the PE p-state ramp happens off the critical path
            warm = consts.tile([P, N_SUB], dt.bfloat16, tag="warm")
            nc.vector.memset(warm[:], 0.0)
            ps_w = psum_pool.tile([P, N_SUB], dt.float32, tag="psum", name="ps_w")
            for _ in range(8):
                nc.tensor.matmul(
                    ps_w[:], warm[:, :P], warm[:], start=True, stop=True
                )

            # W resident, bf16, 8 ko-groups of 1MB. The bandwidth cap is
            # per-HWDGE-queue (~180GB/s each), so split traffic across BOTH
            # HWDGE queues; atm0 and g0 (which together gate the first
            # matmul) go FIRST on opposite queues, g0 in two 0.5MB ko-halves
            # so matmuls start ~3us earlier. Pair 0 consumes groups in
            # merged arrival order CONSUME0 below.
            atms = []
            for t in range(M_TILES):
                atm = at_pool.tile([P, KO, P], dt.bfloat16, tag=f"atm{t}", name=f"atm{t}")
                atms.append(atm)

            def load_atm(t, eng):
                eng.dma_start(atms[t][:], at.ap()[t])

            load_atm(0, nc.sync)
            load_atm(1, nc.sync)

            wgs = [None] * KG
            for g in range(KG):
                wgs[g] = wg_pool.tile(
                    [P, KGS, N], dt.bfloat16, tag=f"wg{g}", name=f"wg{g}"
                )
            # scalar: g0 (2 halves), g1, g4, g5, g7; sync: g2, g3, (g6 later)
            for kk in range(KGS):
                nc.scalar.dma_start(
                    wgs[0][:, kk : kk + 1, :], w.ap()[0, :, kk : kk + 1, :]
                )
            nc.sync.dma_start(wgs[2][:], w.ap()[2])
            nc.scalar.dma_start(wgs[1][:], w.ap()[1])
            nc.sync.dma_start(wgs[3][:], w.ap()[3])
            nc.scalar.dma_start(wgs[4][:], w.ap()[4])
            nc.scalar.dma_start(wgs[5][:], w.ap()[5])
            nc.scalar.dma_start(wgs[7][:], w.ap()[7])
            CONSUME0 = [0, 1, 2, 3, 4, 5, 7, 6]

            # exact fp32 condition: d = (A[m-tile] @ w0 + b0 > 0.5) ? +1 : -1
            # All on DVE, issued inline; PSUM ping-pong below keeps these
            # off every critical path.
            def cond_m(m, dma_eng, dep_inst=None):
                # alternating tags -> distinct semaphores, so cond m's mult
                # never waits on a later a_nat DMA via sem coarsening
                a_nat = anat_pool.tile(
                    [P, K], dt.float32, tag=f"a_nat_{m % 2}", name="a_nat"
                )
                dma_eng.dma_start(a_nat[:], a.ap()[m * P : (m + 1) * P, :])
                scratch = scr_pool.tile([P, K], dt.float32, tag="scratch", name="scratch")
                mult_inst = nc.vector.tensor_tensor(
                    scratch[:], a_nat[:], w0b[:], Alu.mult
                )
                if dep_inst is not None:
                    # hard-order the cond chain after the stts that free the
                    # PSUM banks two m-tiles back: the Tile scheduler orders
                    # the in-order DVE queue by modeled ready time, and a
                    # data-starved cond ahead of an epilogue stt stalls the PE
                    add_dep_helper(
                        mult_inst.ins, dep_inst.ins, reason="conds after bank stts"
                    )
                c0 = d_pool.tile([P, 1], dt.float32, tag=f"c0_{m}", name=f"c0_{m}")
                nc.vector.tensor_reduce(
                    c0[:], scratch[:], mybir.AxisListType.X, Alu.add
                )
                g = d_pool.tile([P, 1], dt.float32, tag=f"g_{m}", name=f"g_{m}")
                nc.vector.tensor_scalar(
                    g[:], c0[:], b128[:, 0:1], 0.5, Alu.add, Alu.is_gt
                )
                d = d_pool.tile([P, 1], dt.float32, tag=f"d_{m}", name=f"d_{m}")
                nc.vector.tensor_scalar(d[:], g[:], 2.0, -1.0, Alu.mult, Alu.add)
                return d

            # conds 0/1 on sync interleaved with the rest of sync's payload
            # (anat0 before g6 so d0 exists before pair 0's psums close);
            # remaining atm tiles spread across both queues
            conds = {}
            conds[0] = cond_m(0, nc.sync)
            conds[1] = cond_m(1, nc.sync)
            nc.sync.dma_start(wgs[6][:], w.ap()[6])
            load_atm(2, nc.sync)
            load_atm(3, nc.sync)
            load_atm(4, nc.sync)
            load_atm(5, nc.sync)
            load_atm(6, nc.scalar)
            load_atm(7, nc.scalar)

            def epilogue(psum, d, m, nq):
                out_sb = out_pool.tile([P, N_SUB], dt.float32, tag="out_sb", name="out_sb")
                stt = nc.vector.scalar_tensor_tensor(
                    out_sb[:],
                    psum[:],
                    d[:],
                    b128[:, nq * N_SUB : (nq + 1) * N_SUB],
                    Alu.add,
                    Alu.add,
                )
                # stores alternate queues by m parity so the final m-tile's
                # stores don't serialize behind the previous tile's
                store_eng = nc.sync if m % 2 == 0 else nc.scalar
                store_eng.dma_start(
                    out.ap()[m * P : (m + 1) * P, nq * N_SUB : (nq + 1) * N_SUB],
                    out_sb[:],
                )
                return stt

            # pair 0: ko-group outer in DMA arrival order, streaming right
            # behind the W group DMAs, both m-tiles across all 8 PSUM banks
            psums = [
                [
                    psum_pool.tile(
                        [P, N_SUB], dt.float32, name=f"ps_{mi}_{nq}", tag="psum"
                    )
                    for nq in range(NQ)
                ]
                for mi in range(2)
            ]
            for gi, g4 in enumerate(CONSUME0):
                for kk in range(KGS):
                    ko = g4 * KGS + kk
                    for mi in range(2):
                        for nq in range(NQ):
                            nc.tensor.matmul(
                                psums[mi][nq][:],
                                atms[mi][:, ko, :],
                                wgs[g4][:, kk, nq * N_SUB : (nq + 1) * N_SUB],
                                start=(gi == 0 and kk == 0),
                                stop=(gi == KG - 1 and kk == KGS - 1),
                            )
            stts_by_m = {}
            for mi in range(2):
                stts_by_m[mi] = [
                    epilogue(psums[mi][nq], conds[mi], mi, nq) for nq in range(NQ)
                ]

            # m-tiles 2..7 SINGLY: 4 psums each, so consecutive m-tiles
            # ping-pong between bank halves and an epilogue stt has a full
            # m-tile period (~14us) before its bank is needed again — late
            # conds can never stall the PE. Their cond chains are pushed to
            # LOW scheduler priority so the Tile scheduler slots them after
            # earlier epilogues in the in-order DVE queue.
            for m in range(2, M_TILES):
                d = cond_m(m, nc.scalar, dep_inst=stts_by_m[m - 2][-1])
                stts_by_m[m] = []
                for nq in range(NQ):
                    psum = psum_pool.tile(
                        [P, N_SUB], dt.float32, name="ps", tag="psum"
                    )
                    for ko in range(KO):
                        nc.tensor.matmul(
                            psum[:],
                            atms[m][:, ko, :],
                            wgs[ko // KGS][:, ko % KGS, nq * N_SUB : (nq + 1) * N_SUB],
                            start=(ko == 0),
                            stop=(ko == KO - 1),
                        )
                    stts_by_m[m].append(epilogue(psum, d, m, nq))

    nc.compile()
    return nc


_NC = None


def _get_nc():
    global _NC
    if _NC is None:
        _NC = _build()
    return _NC


def build_in_maps(a, w, b):
    """Host-side prep: shard A, pre-transpose/block to bf16, cast W to bf16."""
    a = np.ascontiguousarray(a, dtype=np.float32)
    w = np.ascontiguousarray(w, dtype=np.float32)
    b = np.ascontiguousarray(b, dtype=np.float32)
    # w_blk[g, kp, kk, n] = W[(g*KGS+kk)*P + kp, n], bf16
    w_blk = np.ascontiguousarray(
        np.transpose(w.astype(BF16).reshape(KG, KGS, P, N), (0, 2, 1, 3))
    )
    w0 = np.ascontiguousarray(w[:, 0].reshape(1, K))
    in_maps = []
    for i in range(N_CORES):
        a_sh = a[i * M_SHARD : (i + 1) * M_SHARD]
        # at[t, kp, ko, mm] = a_sh[t*128+mm, ko*128+kp]
        at = np.transpose(
            a_sh.reshape(M_TILES, P, KO, P), (0, 3, 2, 1)
        ).astype(BF16)
        in_maps.append(
            {"inputs": a_sh, "at": at, "w": w_blk, "b": b, "w0": w0}
        )
    return in_maps


def kernel(**inputs: np.ndarray) -> np.ndarray:
    a = inputs["inputs"]
    assert a.shape == (B_FULL, K), a.shape
    nc = _get_nc()
    in_maps = build_in_maps(a, inputs["w"], inputs["b"])
    res = run_bass_kernel_spmd(nc, in_maps, core_ids=list(range(N_CORES)))
    return np.concatenate([res.results[i]["out"] for i in range(N_CORES)], axis=0)
